# revision 32
# baseline (speedup 1.0000x reference)
"""EveryStepLoss kernel for Trainium2 (8 NeuronCores, raw Bass).

Reference computation (B=64 segments x L=2048 tokens, C=1024 classes):
    loss[t] = -log_softmax(outputs[t])[targets[t]]          (per-token CE)
    w[t]    = per-segment softmax of linspace(-gamma, gamma, L)
    result  = dot(loss, w) / B

Strategy (v4, see _build_bass_v4):
  - Data-parallel over tokens: core c gets tokens [c*16384, (c+1)*16384)
    (= 8 whole segments, so segments never straddle cores).
  - Per core: one streaming pass over its 64 MiB shard. The per-NC HBM
    cap is ~358 GB/s (187us floor) but the observed rate is
    opportunistic, 340-400 GB/s depending on how loaded the partner NC
    on the shared HBM stack is => stream takes 170-197us run-to-run.
  - exp on ScalarE per whole [128, 4096] tile (~3.7us), per-token row
    sums on VectorE (~4.4us) — both under the 5.3-5.6us/tile DMA pace.
    The last 2 MiB is streamed as 4 x 512 KiB DMAs whose exp+row-sum
    run as ONE fused activation(accum_out=...) each, so the post-stream
    scalar chain is ~1.5us instead of ~9us.
  - x[t, tgt[t]] is fetched two ways (v5). 109 columns use GpSimd
    indirect DMAs (one offset per partition per instruction); their HBM
    random-read service rate is ~84-91 reads/us healthy / ~74 degraded,
    so the gather wall is ~(instrs x 128)/rate from its ~9us start —
    goff is loaded by the GpSimd SWDGE itself as its first instruction
    (an ACT-ring goff starves behind the stream until ~19us). The other
    19 columns (tiles 1,3,..25 and 2,6,..22, runs <= 3) are extracted from the
    already-exp'd tile in SBUF by ONE DVE scalar_tensor_tensor op each:
    accum = sum((iota==tgt[p]) * exp_tile) = exp(x_tgt), using VectorE's
    slack under the stream; a single strided Ln recovers x_tgt. This
    keeps the total gather wall below the stream end in every fabric
    regime measured. (A full SBUF-side select is impossible: ap_gather
    Q7 ucode doesn't compile here — walrus rejects Pool extended-ISA —
    and DVE/GpSimd standard-op selects for all 128 columns would need
    ~137us/~219us, over their budgets.)
  - Final: Ln over the 124 early columns as soon as their sums land,
    Ln over the last 4 at the end; diff = lse - x_tgt, prod = diff*w,
    X-reduce on DVE; TensorE ones-matmul collapses [128,1] -> [1,1]
    PSUM so the output store is a single 4-byte descriptor (a [128,1]
    store's 16 receipts dribble ~7us).
  - The weights w depend only on `lengths` and `gamma` (64 ints + 1
    scalar), so they are precomputed on host and the device computes
    the weighted dot; the 8 per-core scalars are summed on host.
  - Measured window also contains fixed overhead we cannot remove:
    ~6.5us NEFF preamble (excluded from the useful window), and a
    ~7us end-of-NEFF semaphore-teardown storm (54 walrus-reserved sems
    x 5 engines) that IS inside the measured window.
  - Relative error ~1.3e-7 vs the jax reference. HW exec ~189.4us on a
    healthy fabric (189448/189548 measured back-to-back); degraded
    fabric states (engine clocks -15%, gather service -20%) push any
    variant of this kernel to ~215-240.
  - KNOWN DEAD END (do not retry without new information): splitting
    the last TWO 2 MiB tiles into 8 x 512 KiB fused-act tails (NBIG=30,
    NSML=8) deadlocks on HW and in CoreSim — the SP ring issues all 38
    DMAs, but one tail DMA (the 5th use of its slot) never delivers its
    16 completion incs, wedging scalar->vector->PE->store. Slot-count
    and s_red wait formulas were verified correct; the failure is in
    the DMA completion path with 8 outstanding tail DMAs, cause
    undiagnosed. The shipped 4-tail layout (NBIG=31) is stable.
"""

import json

import numpy as np

import concourse.bass as bass
import concourse.mybir as mybir
import concourse.tile as tile
from concourse.bass_utils import run_bass_kernel_spmd

# Problem dims (hardcoded per contract)
B, L, C = 64, 2048, 1024
T = B * L            # 131072 tokens
NCORES = 8
TS = T // NCORES     # 16384 tokens per core
P = 128              # SBUF partitions
Q = 4                # tokens per partition per DMA tile (2 MiB tiles)
SUBQ = 2             # tokens per exp/reduce op ([128, 2048] chunks)
NTILES = TS // (P * Q)   # 32 DMA tiles per core
NCOL = TS // P           # 128 columns of per-token stats

import os as _os

# "v4" (default): raw-bass hybrid — whole-tile exp on ScalarE + row sums
# on VectorE, SWDGE-first goff so the indirect gather starts ~9us,
# 4 x 512KiB tail DMAs with fused exp+sum, minimal final chain.
# "v3": ap_gather variant (does NOT compile on this neuronxcc path).
# "v2"/"raw"/"tile": previous variants.
VARIANT = _os.environ.get("ESL_KERNEL_VARIANT", "v5")

_cached = None       # (nc) built once per process
last_results = None  # BassKernelResults of the most recent run (for test.py)


def _build_bass_v2():
    """Raw-bass variant tuned for the measured-window tail:

    - ScalarE activation(Exp, accum_out=...) computes exp AND the
      per-token row sum in ONE pass per [128, 1024] token-column
      (VectorE does no streaming work at all), so the post-stream
      critical path is one 1.07us fused op instead of the old
      exp(2us)+exp(2us)+reduce(2.3us)+reduce(2.3us) chain.
    - The last 2 MiB of the stream is issued as 4 separate 512 KiB
      DMAs so compute can start on the first quarter while the rest
      streams; everything before that uses 2 MiB DMAs.
    - goff/wt loads go out on the ACT HWDGE ring (nc.scalar.dma_start)
      so the SP ring's very first instruction is stream tile 0, and the
      gathers (service-rate-bound at ~90 reads/us => ~182us total) start
      as early as possible.
    - Final dot: Ln on [128,128] sums, diff/prod/reduce on DVE, then a
      TensorE ones-matmul collapses [128,1] -> [1,1] in PSUM so the
      output store is a single 4-byte descriptor (a [128,1] store's 16
      per-engine receipts were measured to dribble ~7us).
    """
    from contextlib import ExitStack

    nc = bass.Bass()
    x = nc.declare_dram_parameter("x", [TS, C], mybir.dt.float32, isOutput=False)
    goff = nc.declare_dram_parameter("goff", [P, NCOL], mybir.dt.int32, isOutput=False)
    wt = nc.declare_dram_parameter("wt", [P, NCOL], mybir.dt.float32, isOutput=False)
    out = nc.declare_dram_parameter("partial", [1, 1], mybir.dt.float32, isOutput=True)

    FT = mybir.dt.float32
    Exp = mybir.ActivationFunctionType.Exp
    Ln = mybir.ActivationFunctionType.Ln
    NSLOT = 8
    NBIG = NTILES - 1          # 31 full 2 MiB tiles
    NACT = 4 * NBIG + 4        # 128 fused exp+sum ops, one per token-column

    with ExitStack() as ctx:
        xbuf = [
            ctx.enter_context(nc.sbuf_tensor(f"xbuf{i}", [P, Q * C], FT))
            for i in range(NSLOT)
        ]
        gofft = ctx.enter_context(nc.sbuf_tensor("gofft_sb", [P, NCOL], mybir.dt.int32))
        wtt = ctx.enter_context(nc.sbuf_tensor("wtt_sb", [P, NCOL], FT))
        xg = ctx.enter_context(nc.sbuf_tensor("xg_sb", [P, NCOL], FT))
        sums = ctx.enter_context(nc.sbuf_tensor("sums_sb", [P, NCOL], FT))
        lse = ctx.enter_context(nc.sbuf_tensor("lse_sb", [P, NCOL], FT))
        diff = ctx.enter_context(nc.sbuf_tensor("diff_sb", [P, NCOL], FT))
        prod = ctx.enter_context(nc.sbuf_tensor("prod_sb", [P, NCOL], FT))
        partial = ctx.enter_context(nc.sbuf_tensor("partial_sb", [P, 1], FT))
        ones = ctx.enter_context(nc.sbuf_tensor("ones_sb", [P, 1], FT))
        scal = ctx.enter_context(nc.sbuf_tensor("scal_sb", [1, 1], FT))
        scal_ps = ctx.enter_context(nc.psum_tensor("scal_ps", [1, 1], FT))

        s_slot = [ctx.enter_context(nc.semaphore(f"s_slot{i}")) for i in range(NSLOT)]
        s_gin = ctx.enter_context(nc.semaphore("s_gin"))
        s_wt = ctx.enter_context(nc.semaphore("s_wt"))
        s_g = ctx.enter_context(nc.semaphore("s_g"))
        s_act = ctx.enter_context(nc.semaphore("s_act"))
        s_ln = ctx.enter_context(nc.semaphore("s_ln"))
        s_fin = ctx.enter_context(nc.semaphore("s_fin"))
        s_par = ctx.enter_context(nc.semaphore("s_par"))
        s_ones = ctx.enter_context(nc.semaphore("s_ones"))
        s_mm = ctx.enter_context(nc.semaphore("s_mm"))
        s_dve = ctx.enter_context(nc.semaphore("s_dve"))
        s_out = ctx.enter_context(nc.semaphore("s_out"))

        x_tiles = x[:].rearrange("(n p q) c -> n p (q c)", p=P, q=Q)

        with nc.Block() as block:

            @block.sync
            def _(sync):
                # pure stream: 31 x 2 MiB, then 4 x 512 KiB (the last
                # tile's token-columns), so the tail only waits on a
                # quarter tile. Slot i freed once all its token-columns
                # are consumed (s_act counts fused exp+sum ops).
                for j in range(NBIG):
                    if j >= NSLOT:
                        sync.wait_ge(s_act, 4 * (j - NSLOT + 1))
                    sync.dma_start(
                        out=xbuf[j % NSLOT][:], in_=x_tiles[j]
                    ).then_inc(s_slot[j % NSLOT], 16)
                for h in range(4):
                    jj = NBIG + h
                    sync.wait_ge(s_act, 4 * (jj - NSLOT + 1))
                    sync.dma_start(
                        out=xbuf[jj % NSLOT][:, 0:C],
                        in_=x_tiles[NBIG][:, h * C:(h + 1) * C],
                    ).then_inc(s_slot[jj % NSLOT], 16)
                sync.wait_ge(s_dve, 1)
                sync.dma_start(out=out[:], in_=scal[:]).then_inc(s_out, 16)
                sync.wait_ge(s_out, 16)

            @block.scalar
            def _(scalar):
                # goff/wt ride the ACT HWDGE ring; SP ring stays pure
                # stream. goff lands ~8.5us so gathers start immediately.
                scalar.dma_start(out=gofft[:], in_=goff[:]).then_inc(s_gin, 16)
                scalar.dma_start(out=wtt[:], in_=wt[:]).then_inc(s_wt, 16)
                for j in range(NBIG):
                    scalar.wait_ge(s_slot[j % NSLOT], 16 * (j // NSLOT + 1))
                    for qq in range(Q):
                        sl = slice(qq * C, (qq + 1) * C)
                        scalar.activation(
                            out=xbuf[j % NSLOT][:, sl],
                            in_=xbuf[j % NSLOT][:, sl],
                            func=Exp,
                            accum_out=sums[:, Q * j + qq:Q * j + qq + 1],
                        ).then_inc(s_act, 1)
                # early Ln on the columns whose sums are complete
                # (self-wait: accum_out write -> read needs explicit sync)
                scalar.wait_ge(s_act, 4 * NBIG)
                scalar.activation(
                    out=lse[:, 0:4 * NBIG], in_=sums[:, 0:4 * NBIG], func=Ln
                ).then_inc(s_ln, 1)
                for h in range(4):
                    jj = NBIG + h
                    scalar.wait_ge(s_slot[jj % NSLOT], 16 * (jj // NSLOT + 1))
                    scalar.activation(
                        out=xbuf[jj % NSLOT][:, 0:C],
                        in_=xbuf[jj % NSLOT][:, 0:C],
                        func=Exp,
                        accum_out=sums[:, 4 * NBIG + h:4 * NBIG + h + 1],
                    ).then_inc(s_act, 1)
                scalar.wait_ge(s_act, NACT)
                scalar.activation(
                    out=lse[:, 4 * NBIG:NCOL], in_=sums[:, 4 * NBIG:NCOL], func=Ln
                ).then_inc(s_ln, 1)

            @block.gpsimd
            def _(gpsimd):
                gpsimd.memset(ones[:], 1.0).then_inc(s_ones, 1)
                gpsimd.wait_ge(s_gin, 16)
                for col in range(NCOL):
                    gpsimd.indirect_dma_start(
                        out=xg[:, col:col + 1],
                        out_offset=None,
                        in_=x[:],
                        in_offset=bass.IndirectOffsetOnAxis(
                            ap=gofft[:, col:col + 1], axis=1
                        ),
                    ).then_inc(s_g, 16)

            @block.vector
            def _(vector):
                # all waits here resolve by ~stream end; the only tail
                # ops are the 4-column diff/prod and the final reduce.
                vector.wait_ge(s_ln, 1)
                vector.wait_ge(s_g, 16 * NCOL)
                vector.wait_ge(s_wt, 16)
                vector.tensor_tensor(
                    out=diff[:, 0:4 * NBIG], in0=lse[:, 0:4 * NBIG],
                    in1=xg[:, 0:4 * NBIG], op=mybir.AluOpType.subtract,
                ).then_inc(s_fin, 1)
                vector.wait_ge(s_fin, 1)
                vector.tensor_tensor(
                    out=prod[:, 0:4 * NBIG], in0=diff[:, 0:4 * NBIG],
                    in1=wtt[:, 0:4 * NBIG], op=mybir.AluOpType.mult,
                ).then_inc(s_fin, 1)
                vector.wait_ge(s_ln, 2)
                vector.tensor_tensor(
                    out=diff[:, 4 * NBIG:NCOL], in0=lse[:, 4 * NBIG:NCOL],
                    in1=xg[:, 4 * NBIG:NCOL], op=mybir.AluOpType.subtract,
                ).then_inc(s_fin, 1)
                vector.wait_ge(s_fin, 3)
                vector.tensor_tensor(
                    out=prod[:, 4 * NBIG:NCOL], in0=diff[:, 4 * NBIG:NCOL],
                    in1=wtt[:, 4 * NBIG:NCOL], op=mybir.AluOpType.mult,
                ).then_inc(s_fin, 1)
                vector.wait_ge(s_fin, 4)
                vector.tensor_reduce(
                    out=partial[:],
                    in_=prod[:],
                    axis=mybir.AxisListType.X,
                    op=mybir.AluOpType.add,
                ).then_inc(s_par, 1)
                vector.wait_ge(s_mm, 1)
                vector.tensor_copy(out=scal[:], in_=scal_ps[:]).then_inc(s_dve, 1)

            @block.tensor
            def _(tensor):
                tensor.wait_ge(s_ones, 1)
                tensor.wait_ge(s_par, 1)
                tensor.matmul(
                    scal_ps[:], partial[:], ones[:], start=True, stop=True,
                ).then_inc(s_mm, 1)

    return nc


def _build_bass():
    nc = bass.Bass()
    x = nc.declare_dram_parameter("x", [TS, C], mybir.dt.float32, isOutput=False)
    goff = nc.declare_dram_parameter("goff", [P, NCOL], mybir.dt.int32, isOutput=False)
    wt = nc.declare_dram_parameter("wt", [P, NCOL], mybir.dt.float32, isOutput=False)
    out = nc.declare_dram_parameter("partial", [1, 1], mybir.dt.float32, isOutput=True)

    FT = mybir.dt.float32
    Exp = mybir.ActivationFunctionType.Exp
    Ln = mybir.ActivationFunctionType.Ln

    with tile.TileContext(nc) as tc:
        with (
            tc.tile_pool(name="xp", bufs=5) as xp,
            tc.tile_pool(name="small", bufs=1) as small,
            tc.tile_pool(name="ps", bufs=1, space="PSUM") as psp,
        ):
            gofft = small.tile([P, NCOL], mybir.dt.int32)
            wtt = small.tile([P, NCOL], FT)
            xg = small.tile([P, NCOL], FT)
            sums = small.tile([P, NCOL], FT)
            lse = small.tile([P, NCOL], FT)
            diff = small.tile([P, NCOL], FT)
            prod = small.tile([P, NCOL], FT)
            partial = small.tile([P, 1], FT)

            nc.sync.dma_start(out=gofft[:], in_=goff[:])

            # Gather x[t, tgt[t]]. Offsets are flat element indices
            # t*C + tgt[t], laid out to match the [partition, column] token
            # layout below. HW indirect DMA consumes ONE offset per
            # partition (contiguous run = dest row size), so gather one
            # column (128 tokens) per instruction.
            for col in range(NCOL):
                nc.gpsimd.indirect_dma_start(
                    out=xg[:, col:col + 1],
                    out_offset=None,
                    in_=x[:],
                    in_offset=bass.IndirectOffsetOnAxis(
                        ap=gofft[:, col:col + 1], axis=1
                    ),
                )

            # Token layout: DMA tile j ([128, 4096] = 2 MiB), partition p,
            # sub-slot qq in 0..3  <->  token t_local = 512*j + 4*p + qq;
            # stats column = 4*j + qq. Exp on ScalarE and row-sums on
            # VectorE both run on [128, 2048] half-tiles so the end-of-
            # stream latency stays small; both engines stay under the
            # ~185us DMA stream.
            x_tiles = x[:].rearrange("(n p q) c -> n p (q c)", p=P, q=Q)
            for j in range(NTILES):
                xt = xp.tile([P, Q * C], FT)
                nc.sync.dma_start(out=xt[:], in_=x_tiles[j])
                for h in range(Q // SUBQ):
                    sl = slice(h * SUBQ * C, (h + 1) * SUBQ * C)
                    nc.scalar.activation(out=xt[:, sl], in_=xt[:, sl], func=Exp)
                    nc.vector.tensor_reduce(
                        out=sums[:, Q * j + h * SUBQ:Q * j + (h + 1) * SUBQ],
                        in_=xt[:, sl].rearrange("p (q c) -> p q c", q=SUBQ),
                        axis=mybir.AxisListType.X,
                        op=mybir.AluOpType.add,
                    )

            nc.sync.dma_start(out=wtt[:], in_=wt[:])
            nc.scalar.activation(out=lse[:], in_=sums[:], func=Ln)
            nc.vector.tensor_tensor(
                out=diff[:], in0=lse[:], in1=xg[:], op=mybir.AluOpType.subtract
            )
            nc.vector.tensor_tensor(
                out=prod[:], in0=diff[:], in1=wtt[:], op=mybir.AluOpType.mult
            )
            nc.vector.tensor_reduce(
                out=partial[:],
                in_=prod[:],
                axis=mybir.AxisListType.X,
                op=mybir.AluOpType.add,
            )
            # Cross-partition reduce on the (idle) TensorE so the output
            # store is a single 4-byte descriptor — a [128, 1] store's 16
            # per-engine completion receipts were measured to dribble in
            # over ~6us at kernel end.
            ones = small.tile([P, 1], FT)
            nc.gpsimd.memset(ones[:], 1.0)
            scal_ps = psp.tile([1, 1], FT)
            nc.tensor.matmul(
                out=scal_ps[:], lhsT=partial[:], rhs=ones[:], start=True, stop=True
            )
            scal = small.tile([1, 1], FT)
            nc.vector.tensor_copy(out=scal[:], in_=scal_ps[:])
            nc.sync.dma_start(out=out[:], in_=scal[:])
    return nc


def _build_bass_v3():
    """Raw-bass variant with NO indirect-DMA gather.

    The old design fetched x[t, tgt[t]] with 128 GpSimd indirect DMAs;
    their HBM random-read service rate is ~90 reads/us, a fixed ~183us
    => the gather, not the stream, set the kernel end on fast runs.
    Here the target logits are extracted from SBUF while each tile is
    resident, BEFORE exp overwrites it, using the GpSimd ap_gather
    ucode (out[p, j'] = src[p, unwrapped_idx[j']], idx shared per
    16-partition group). For a [128, 4q] idx block, unwrapped position
    j' = r*16 + (p%16) holds src[p, r*1024 + tgt] — i.e. each token's
    target sits at a fixed per-partition diagonal, which one fused
    tensor_tensor_reduce (mult by a 0/1 mask, add-reduce over 16)
    collapses into xg[:, 4j+r].

    Other structure: 31 x 2MiB stream DMAs + 4 x 512KiB for the last
    tile (short tail); exp on ScalarE per whole tile ([128,4096]);
    per-token row sums on VectorE; the 4 tail tiles use the fused
    activation accum_out (exp+sum in one op) so the post-stream chain
    is minimal; final dot -> TensorE ones-matmul -> single 4B store.
    Small tables (idx/mask/wt) ride the ACT HWDGE ring.
    """
    from contextlib import ExitStack

    nc = bass.Bass()
    x = nc.declare_dram_parameter("x", [TS, C], mybir.dt.float32, isOutput=False)
    gidx = nc.declare_dram_parameter("gidx", [P, NCOL], mybir.dt.int16, isOutput=False)
    mask = nc.declare_dram_parameter("mask", [P, 64], mybir.dt.float32, isOutput=False)
    wt = nc.declare_dram_parameter("wt", [P, NCOL], mybir.dt.float32, isOutput=False)
    out = nc.declare_dram_parameter("partial", [1, 1], mybir.dt.float32, isOutput=True)

    FT = mybir.dt.float32
    Exp = mybir.ActivationFunctionType.Exp
    Ln = mybir.ActivationFunctionType.Ln
    Ident = mybir.ActivationFunctionType.Identity
    NSLOT = 8
    NBIG = NTILES - 1          # 31 full 2 MiB tiles

    with ExitStack() as ctx:
        xbuf = [
            ctx.enter_context(nc.sbuf_tensor(f"xbuf{i}", [P, Q * C], FT))
            for i in range(NSLOT)
        ]
        gidxt = ctx.enter_context(nc.sbuf_tensor("gidxt_sb", [P, NCOL], mybir.dt.int16))
        maskt = ctx.enter_context(nc.sbuf_tensor("maskt_sb", [P, 64], FT))
        wtt = ctx.enter_context(nc.sbuf_tensor("wtt_sb", [P, NCOL], FT))
        og = [
            ctx.enter_context(nc.sbuf_tensor(f"og{i}_sb", [P, 64], FT))
            for i in range(2)
        ]
        ogs = [
            ctx.enter_context(nc.sbuf_tensor(f"ogs{i}_sb", [P, 16], FT))
            for i in range(4)
        ]
        trash = [
            ctx.enter_context(nc.sbuf_tensor(f"trash{i}_sb", [P, 64], FT))
            for i in range(2)
        ]
        xg = ctx.enter_context(nc.sbuf_tensor("xg_sb", [P, NCOL], FT))
        sums = ctx.enter_context(nc.sbuf_tensor("sums_sb", [P, NCOL], FT))
        lse = ctx.enter_context(nc.sbuf_tensor("lse_sb", [P, NCOL], FT))
        diff = ctx.enter_context(nc.sbuf_tensor("diff_sb", [P, NCOL], FT))
        prod = ctx.enter_context(nc.sbuf_tensor("prod_sb", [P, NCOL], FT))
        partial = ctx.enter_context(nc.sbuf_tensor("partial_sb", [P, 1], FT))
        ones = ctx.enter_context(nc.sbuf_tensor("ones_sb", [P, 1], FT))
        scal = ctx.enter_context(nc.sbuf_tensor("scal_sb", [1, 1], FT))
        scal_ps = ctx.enter_context(nc.psum_tensor("scal_ps", [1, 1], FT))

        s_slot = [ctx.enter_context(nc.semaphore(f"s_slot{i}")) for i in range(NSLOT)]
        s_idx = ctx.enter_context(nc.semaphore("s_idx"))
        s_mask = ctx.enter_context(nc.semaphore("s_mask"))
        s_wt = ctx.enter_context(nc.semaphore("s_wt"))
        s_apg = ctx.enter_context(nc.semaphore("s_apg"))
        s_act = ctx.enter_context(nc.semaphore("s_act"))
        s_red = ctx.enter_context(nc.semaphore("s_red"))
        s_dvem = ctx.enter_context(nc.semaphore("s_dvem"))
        s_ln = ctx.enter_context(nc.semaphore("s_ln"))
        s_fin = ctx.enter_context(nc.semaphore("s_fin"))
        s_par = ctx.enter_context(nc.semaphore("s_par"))
        s_ones = ctx.enter_context(nc.semaphore("s_ones"))
        s_mm = ctx.enter_context(nc.semaphore("s_mm"))
        s_dve = ctx.enter_context(nc.semaphore("s_dve"))
        s_out = ctx.enter_context(nc.semaphore("s_out"))

        x_tiles = x[:].rearrange("(n p q) c -> n p (q c)", p=P, q=Q)

        with nc.Block() as block:

            @block.sync
            def _(sync):
                # pure stream; slot i freed once vector's row-sum of its
                # previous tile is done (reduce happens after apg+exp).
                for j in range(NBIG):
                    if j >= NSLOT:
                        sync.wait_ge(s_red, j - NSLOT + 1)
                    sync.dma_start(
                        out=xbuf[j % NSLOT][:], in_=x_tiles[j]
                    ).then_inc(s_slot[j % NSLOT], 16)
                for h in range(4):
                    jj = NBIG + h
                    sync.wait_ge(s_red, jj - NSLOT + 1)
                    sync.dma_start(
                        out=xbuf[jj % NSLOT][:, 0:C],
                        in_=x_tiles[NBIG][:, h * C:(h + 1) * C],
                    ).then_inc(s_slot[jj % NSLOT], 16)
                sync.wait_ge(s_dve, 1)
                sync.dma_start(out=out[:], in_=scal[:]).then_inc(s_out, 16)
                sync.wait_ge(s_out, 16)

            @block.scalar
            def _(scalar):
                scalar.dma_start(out=gidxt[:], in_=gidx[:]).then_inc(s_idx, 16)
                scalar.dma_start(out=maskt[:], in_=mask[:]).then_inc(s_mask, 16)
                scalar.dma_start(out=wtt[:], in_=wt[:]).then_inc(s_wt, 16)
                # ones = garbage*0 + 1 (interp skips the uninit check when
                # scale is an immediate 0)
                scalar.activation(
                    out=ones[:], in_=ones[:], func=Ident, bias=1.0, scale=0.0
                ).then_inc(s_ones, 1)
                for j in range(NBIG):
                    scalar.wait_ge(s_slot[j % NSLOT], 16 * (j // NSLOT + 1))
                    scalar.wait_ge(s_apg, j + 1)   # apg reads x before exp
                    scalar.activation(
                        out=xbuf[j % NSLOT][:], in_=xbuf[j % NSLOT][:], func=Exp
                    ).then_inc(s_act, 1)
                scalar.wait_ge(s_red, NBIG)
                scalar.activation(
                    out=lse[:, 0:4 * NBIG], in_=sums[:, 0:4 * NBIG], func=Ln
                ).then_inc(s_ln, 1)
                for h in range(4):
                    jj = NBIG + h
                    scalar.wait_ge(s_slot[jj % NSLOT], 16 * (jj // NSLOT + 1))
                    scalar.wait_ge(s_apg, NBIG + h + 1)
                    scalar.activation(
                        out=xbuf[jj % NSLOT][:, 0:C],
                        in_=xbuf[jj % NSLOT][:, 0:C],
                        func=Exp,
                        accum_out=sums[:, 4 * NBIG + h:4 * NBIG + h + 1],
                    ).then_inc(s_act, 1)
                scalar.wait_ge(s_act, NBIG + 4)
                scalar.activation(
                    out=lse[:, 4 * NBIG:NCOL], in_=sums[:, 4 * NBIG:NCOL], func=Ln
                ).then_inc(s_ln, 1)

            @block.gpsimd
            def _(gpsimd):
                from concourse import library_config

                gpsimd.load_library(library_config.ap_gather)
                gpsimd.wait_ge(s_idx, 16)
                for j in range(NBIG):
                    gpsimd.wait_ge(s_slot[j % NSLOT], 16 * (j // NSLOT + 1))
                    if j >= 2:
                        gpsimd.wait_ge(s_dvem, j - 1)   # og[j%2] free
                    gpsimd.ap_gather(
                        out_ap=og[j % 2][:],
                        in_ap=xbuf[j % NSLOT][:],
                        idxs_ap=gidxt[:, Q * j:Q * j + Q],
                        channels=P,
                        num_elems=Q * C,
                        d=1,
                        num_idxs=64,
                    ).then_inc(s_apg, 1)
                for h in range(4):
                    jj = NBIG + h
                    gpsimd.wait_ge(s_slot[jj % NSLOT], 16 * (jj // NSLOT + 1))
                    gpsimd.ap_gather(
                        out_ap=ogs[h][:],
                        in_ap=xbuf[jj % NSLOT][:, 0:C],
                        idxs_ap=gidxt[:, 4 * NBIG + h:4 * NBIG + h + 1],
                        channels=P,
                        num_elems=C,
                        d=1,
                        num_idxs=16,
                    ).then_inc(s_apg, 1)

            @block.vector
            def _(vector):
                vector.wait_ge(s_mask, 16)
                for j in range(NBIG):
                    vector.wait_ge(s_act, j + 1)
                    vector.tensor_reduce(
                        out=sums[:, Q * j:Q * j + Q],
                        in_=xbuf[j % NSLOT][:].rearrange("p (q c) -> p q c", q=Q),
                        axis=mybir.AxisListType.X,
                        op=mybir.AluOpType.add,
                    ).then_inc(s_red, 1)
                    vector.wait_ge(s_apg, j + 1)
                    for r in range(Q):
                        ib = vector.tensor_tensor_reduce(
                            out=trash[j % 2][:, 16 * r:16 * r + 16],
                            in0=og[j % 2][:, 16 * r:16 * r + 16],
                            in1=maskt[:, 16 * r:16 * r + 16],
                            scale=1.0,
                            scalar=0.0,
                            op0=mybir.AluOpType.mult,
                            op1=mybir.AluOpType.add,
                            accum_out=xg[:, Q * j + r:Q * j + r + 1],
                        )
                        if r == Q - 1:
                            ib.then_inc(s_dvem, 1)
                # early diff/prod for the columns already complete
                vector.wait_ge(s_dvem, NBIG)   # own accum writes -> sync
                vector.wait_ge(s_ln, 1)
                vector.wait_ge(s_wt, 16)
                vector.tensor_tensor(
                    out=diff[:, 0:4 * NBIG], in0=lse[:, 0:4 * NBIG],
                    in1=xg[:, 0:4 * NBIG], op=mybir.AluOpType.subtract,
                ).then_inc(s_fin, 1)
                vector.wait_ge(s_fin, 1)
                vector.tensor_tensor(
                    out=prod[:, 0:4 * NBIG], in0=diff[:, 0:4 * NBIG],
                    in1=wtt[:, 0:4 * NBIG], op=mybir.AluOpType.mult,
                ).then_inc(s_fin, 1)
                for h in range(4):
                    vector.wait_ge(s_apg, NBIG + h + 1)
                    vector.tensor_tensor_reduce(
                        out=trash[0][:, 16 * h:16 * h + 16],
                        in0=ogs[h][:],
                        in1=maskt[:, 0:16],
                        scale=1.0,
                        scalar=0.0,
                        op0=mybir.AluOpType.mult,
                        op1=mybir.AluOpType.add,
                        accum_out=xg[:, 4 * NBIG + h:4 * NBIG + h + 1],
                    ).then_inc(s_dvem, 1)
                vector.wait_ge(s_dvem, NBIG + 4)
                vector.wait_ge(s_ln, 2)
                vector.tensor_tensor(
                    out=diff[:, 4 * NBIG:NCOL], in0=lse[:, 4 * NBIG:NCOL],
                    in1=xg[:, 4 * NBIG:NCOL], op=mybir.AluOpType.subtract,
                ).then_inc(s_fin, 1)
                vector.wait_ge(s_fin, 3)
                vector.tensor_tensor(
                    out=prod[:, 4 * NBIG:NCOL], in0=diff[:, 4 * NBIG:NCOL],
                    in1=wtt[:, 4 * NBIG:NCOL], op=mybir.AluOpType.mult,
                ).then_inc(s_fin, 1)
                vector.wait_ge(s_fin, 4)
                vector.tensor_reduce(
                    out=partial[:],
                    in_=prod[:],
                    axis=mybir.AxisListType.X,
                    op=mybir.AluOpType.add,
                ).then_inc(s_par, 1)
                vector.wait_ge(s_mm, 1)
                vector.tensor_copy(out=scal[:], in_=scal_ps[:]).then_inc(s_dve, 1)

            @block.tensor
            def _(tensor):
                tensor.wait_ge(s_ones, 1)
                tensor.wait_ge(s_par, 1)
                tensor.matmul(
                    scal_ps[:], partial[:], ones[:], start=True, stop=True,
                ).then_inc(s_mm, 1)

    return nc


def _build_bass_v4():
    """Raw-bass hybrid (the ap_gather ucode of v3 does not compile on
    this neuronxcc path — walrus visitInstISA rejects Pool extended-ISA,
    so the target fetch stays an indirect-DMA gather, which is HBM
    random-read service-bound at ~90 reads/us => ~186us wall from first
    dispatch; everything is arranged so it starts as early as possible
    and nothing else ever waits on it until the very end):

    - goff is loaded by the GpSimd SWDGE itself as its first
      instruction (lands ~9us; an ACT-ring goff was measured to starve
      behind the stream until ~19us, an SP-ring one delays the stream).
    - Stream: 31 x 2MiB + 4 x 512KiB tail DMAs on the SP ring only.
    - exp on ScalarE per whole tile ([128,4096], ~3.7us); per-token row
      sums on VectorE ([128,4x1024] reduce, ~4.4us); both well under the
      5.3-5.6us/tile DMA pace. The 4 tail tiles use activation
      accum_out (exp+sum fused, 1.4us) so the post-stream scalar chain
      is one op.
    - Final: Ln(124 cols) early / Ln(4) late, diff/prod/reduce on DVE
      after the last gather lands, TensorE ones-matmul -> [1,1] PSUM ->
      DVE copy -> single 4B store.
    """
    from contextlib import ExitStack

    nc = bass.Bass()
    x = nc.declare_dram_parameter("x", [TS, C], mybir.dt.float32, isOutput=False)
    goff = nc.declare_dram_parameter("goff", [P, NCOL], mybir.dt.int32, isOutput=False)
    wt = nc.declare_dram_parameter("wt", [P, NCOL], mybir.dt.float32, isOutput=False)
    out = nc.declare_dram_parameter("partial", [1, 1], mybir.dt.float32, isOutput=True)

    FT = mybir.dt.float32
    Exp = mybir.ActivationFunctionType.Exp
    Ln = mybir.ActivationFunctionType.Ln
    Ident = mybir.ActivationFunctionType.Identity
    NSLOT = 8
    NBIG = NTILES - 1          # 31 full 2 MiB tiles

    with ExitStack() as ctx:
        xbuf = [
            ctx.enter_context(nc.sbuf_tensor(f"xbuf{i}", [P, Q * C], FT))
            for i in range(NSLOT)
        ]
        gofft = ctx.enter_context(nc.sbuf_tensor("gofft_sb", [P, NCOL], mybir.dt.int32))
        wtt = ctx.enter_context(nc.sbuf_tensor("wtt_sb", [P, NCOL], FT))
        xg = ctx.enter_context(nc.sbuf_tensor("xg_sb", [P, NCOL], FT))
        sums = ctx.enter_context(nc.sbuf_tensor("sums_sb", [P, NCOL], FT))
        lse = ctx.enter_context(nc.sbuf_tensor("lse_sb", [P, NCOL], FT))
        diff = ctx.enter_context(nc.sbuf_tensor("diff_sb", [P, NCOL], FT))
        prod = ctx.enter_context(nc.sbuf_tensor("prod_sb", [P, NCOL], FT))
        partial = ctx.enter_context(nc.sbuf_tensor("partial_sb", [P, 1], FT))
        ones = ctx.enter_context(nc.sbuf_tensor("ones_sb", [P, 1], FT))
        scal = ctx.enter_context(nc.sbuf_tensor("scal_sb", [1, 1], FT))
        scal_ps = ctx.enter_context(nc.psum_tensor("scal_ps", [1, 1], FT))

        s_slot = [ctx.enter_context(nc.semaphore(f"s_slot{i}")) for i in range(NSLOT)]
        s_gin = ctx.enter_context(nc.semaphore("s_gin"))
        s_wt = ctx.enter_context(nc.semaphore("s_wt"))
        s_g = ctx.enter_context(nc.semaphore("s_g"))
        s_exp = ctx.enter_context(nc.semaphore("s_exp"))
        s_red = ctx.enter_context(nc.semaphore("s_red"))
        s_sml = ctx.enter_context(nc.semaphore("s_sml"))
        s_ln = ctx.enter_context(nc.semaphore("s_ln"))
        s_fin = ctx.enter_context(nc.semaphore("s_fin"))
        s_par = ctx.enter_context(nc.semaphore("s_par"))
        s_ones = ctx.enter_context(nc.semaphore("s_ones"))
        s_mm = ctx.enter_context(nc.semaphore("s_mm"))
        s_dve = ctx.enter_context(nc.semaphore("s_dve"))
        s_out = ctx.enter_context(nc.semaphore("s_out"))

        x_tiles = x[:].rearrange("(n p q) c -> n p (q c)", p=P, q=Q)

        with nc.Block() as block:

            @block.sync
            def _(sync):
                for j in range(NBIG):
                    if j >= NSLOT:
                        sync.wait_ge(s_red, j - NSLOT + 1)
                    sync.dma_start(
                        out=xbuf[j % NSLOT][:], in_=x_tiles[j]
                    ).then_inc(s_slot[j % NSLOT], 16)
                for h in range(4):
                    jj = NBIG + h
                    sync.wait_ge(s_red, jj - NSLOT + 1)
                    sync.dma_start(
                        out=xbuf[jj % NSLOT][:, 0:C],
                        in_=x_tiles[NBIG][:, h * C:(h + 1) * C],
                    ).then_inc(s_slot[jj % NSLOT], 16)
                sync.wait_ge(s_dve, 1)
                sync.dma_start(out=out[:], in_=scal[:]).then_inc(s_out, 16)
                sync.wait_ge(s_out, 16)

            @block.scalar
            def _(scalar):
                scalar.dma_start(out=wtt[:], in_=wt[:]).then_inc(s_wt, 16)
                # ones = garbage*0 + 1
                scalar.activation(
                    out=ones[:], in_=ones[:], func=Ident, bias=1.0, scale=0.0
                ).then_inc(s_ones, 1)
                for j in range(NBIG):
                    scalar.wait_ge(s_slot[j % NSLOT], 16 * (j // NSLOT + 1))
                    scalar.activation(
                        out=xbuf[j % NSLOT][:], in_=xbuf[j % NSLOT][:], func=Exp
                    ).then_inc(s_exp, 1)
                scalar.wait_ge(s_red, NBIG)
                scalar.activation(
                    out=lse[:, 0:4 * NBIG], in_=sums[:, 0:4 * NBIG], func=Ln
                ).then_inc(s_ln, 1)
                for h in range(4):
                    jj = NBIG + h
                    scalar.wait_ge(s_slot[jj % NSLOT], 16 * (jj // NSLOT + 1))
                    scalar.activation(
                        out=xbuf[jj % NSLOT][:, 0:C],
                        in_=xbuf[jj % NSLOT][:, 0:C],
                        func=Exp,
                        accum_out=sums[:, 4 * NBIG + h:4 * NBIG + h + 1],
                    ).then_inc(s_sml, 1)
                scalar.wait_ge(s_sml, 4)
                scalar.activation(
                    out=lse[:, 4 * NBIG:NCOL], in_=sums[:, 4 * NBIG:NCOL], func=Ln
                ).then_inc(s_ln, 1)

            @block.gpsimd
            def _(gpsimd):
                gpsimd.dma_start(out=gofft[:], in_=goff[:]).then_inc(s_gin, 16)
                gpsimd.wait_ge(s_gin, 16)
                for col in range(NCOL):
                    gpsimd.indirect_dma_start(
                        out=xg[:, col:col + 1],
                        out_offset=None,
                        in_=x[:],
                        in_offset=bass.IndirectOffsetOnAxis(
                            ap=gofft[:, col:col + 1], axis=1
                        ),
                    ).then_inc(s_g, 16)

            @block.vector
            def _(vector):
                for j in range(NBIG):
                    vector.wait_ge(s_exp, j + 1)
                    vector.tensor_reduce(
                        out=sums[:, Q * j:Q * j + Q],
                        in_=xbuf[j % NSLOT][:].rearrange("p (q c) -> p q c", q=Q),
                        axis=mybir.AxisListType.X,
                        op=mybir.AluOpType.add,
                    ).then_inc(s_red, 1)
                vector.wait_ge(s_ln, 1)
                vector.wait_ge(s_g, 16 * NCOL)
                vector.wait_ge(s_wt, 16)
                vector.tensor_tensor(
                    out=diff[:, 0:4 * NBIG], in0=lse[:, 0:4 * NBIG],
                    in1=xg[:, 0:4 * NBIG], op=mybir.AluOpType.subtract,
                ).then_inc(s_fin, 1)
                vector.wait_ge(s_fin, 1)
                vector.tensor_tensor(
                    out=prod[:, 0:4 * NBIG], in0=diff[:, 0:4 * NBIG],
                    in1=wtt[:, 0:4 * NBIG], op=mybir.AluOpType.mult,
                ).then_inc(s_fin, 1)
                vector.wait_ge(s_ln, 2)
                vector.tensor_tensor(
                    out=diff[:, 4 * NBIG:NCOL], in0=lse[:, 4 * NBIG:NCOL],
                    in1=xg[:, 4 * NBIG:NCOL], op=mybir.AluOpType.subtract,
                ).then_inc(s_fin, 1)
                vector.wait_ge(s_fin, 3)
                vector.tensor_tensor(
                    out=prod[:, 4 * NBIG:NCOL], in0=diff[:, 4 * NBIG:NCOL],
                    in1=wtt[:, 4 * NBIG:NCOL], op=mybir.AluOpType.mult,
                ).then_inc(s_fin, 1)
                vector.wait_ge(s_fin, 4)
                vector.tensor_reduce(
                    out=partial[:],
                    in_=prod[:],
                    axis=mybir.AxisListType.X,
                    op=mybir.AluOpType.add,
                ).then_inc(s_par, 1)
                vector.wait_ge(s_mm, 1)
                vector.tensor_copy(out=scal[:], in_=scal_ps[:]).then_inc(s_dve, 1)

            @block.tensor
            def _(tensor):
                tensor.wait_ge(s_ones, 1)
                tensor.wait_ge(s_par, 1)
                tensor.matmul(
                    scal_ps[:], partial[:], ones[:], start=True, stop=True,
                ).then_inc(s_mm, 1)

    return nc



def _build_bass_v5():
    """v4 plus DVE-slack target extraction for 19 of the 128 columns.

    The indirect-DMA gather is HBM random-read service-bound (~84-91
    reads/us healthy, ~74 degraded), so its wall time is ~(128 instrs x
    128 reads)/rate ~ 188-222us — the kernel's critical path in v4.
    VectorE has ~35-50us of slack under the stream, and ONE
    scalar_tensor_tensor op per column computes
        accum = sum((iota == tgt[p]) * exp_tile[p, :]) = exp(x_tgt)
    from the already-exp'd tile while it is still in SBUF (1.07us per
    [128,1024] column). 19 columns (tiles 1,3..25 plus 2,6..22, runs
    of <= 3 consecutive select tiles with recovery gaps) are extracted
    this way and two strided Ln ops turn them back into x_tgt; the
    remaining 109 columns keep the DMA gather, whose wall shrinks ~15%
    to at-or-below the stream end in every fabric regime measured. Select-tile slots are freed by the select op (it is the
    last reader of the tile), gather-tile slots by the row-sum reduce.
    """
    from contextlib import ExitStack

    nc = bass.Bass()
    x = nc.declare_dram_parameter("x", [TS, C], mybir.dt.float32, isOutput=False)
    goff = nc.declare_dram_parameter("goff", [P, NCOL], mybir.dt.int32, isOutput=False)
    wt = nc.declare_dram_parameter("wt", [P, NCOL], mybir.dt.float32, isOutput=False)
    tgtf = nc.declare_dram_parameter("tgtf", [P, NCOL], mybir.dt.float32, isOutput=False)
    iotaf = nc.declare_dram_parameter("iotaf", [P, C], mybir.dt.float32, isOutput=False)
    out = nc.declare_dram_parameter("partial", [1, 1], mybir.dt.float32, isOutput=True)

    FT = mybir.dt.float32
    Exp = mybir.ActivationFunctionType.Exp
    Ln = mybir.ActivationFunctionType.Ln
    Ident = mybir.ActivationFunctionType.Identity
    NSLOT = 8
    NBIG = NTILES - 1          # 31 full 2 MiB tiles
    # A select tile costs reduce(4.4us)+sts(1.1us) ~ 5.5us = the DMA
    # pace, so long runs of consecutive select tiles accumulate lag
    # that cascades into the tail (measured ~9us with 24 consecutive).
    # 19 selects with runs <= 3 and recovery gaps (tiles 4,8,12,16,20,
    # 24, 26..30 free, +1.1us catch-up each) keep VectorE under the
    # stream while cutting the gather to 109 instructions.
    SEL_TILES = sorted(
        set(range(1, 26, 2)) | {2, 6, 10, 14, 18, 22}
    )                                       # 19 select tiles, col = 4*j
    SEL_COLS = {4 * j for j in SEL_TILES}
    NGATH = NCOL - len(SEL_COLS)            # 109 gathered columns

    with ExitStack() as ctx:
        xbuf = [
            ctx.enter_context(nc.sbuf_tensor(f"xbuf{i}", [P, Q * C], FT))
            for i in range(NSLOT)
        ]
        gofft = ctx.enter_context(nc.sbuf_tensor("gofft_sb", [P, NCOL], mybir.dt.int32))
        wtt = ctx.enter_context(nc.sbuf_tensor("wtt_sb", [P, NCOL], FT))
        tgtft = ctx.enter_context(nc.sbuf_tensor("tgtft_sb", [P, NCOL], FT))
        iotat = ctx.enter_context(nc.sbuf_tensor("iotat_sb", [P, C], FT))
        trasht = ctx.enter_context(nc.sbuf_tensor("trasht_sb", [P, C], FT))
        xg = ctx.enter_context(nc.sbuf_tensor("xg_sb", [P, NCOL], FT))
        sums = ctx.enter_context(nc.sbuf_tensor("sums_sb", [P, NCOL], FT))
        lse = ctx.enter_context(nc.sbuf_tensor("lse_sb", [P, NCOL], FT))
        diff = ctx.enter_context(nc.sbuf_tensor("diff_sb", [P, NCOL], FT))
        prod = ctx.enter_context(nc.sbuf_tensor("prod_sb", [P, NCOL], FT))
        partial = ctx.enter_context(nc.sbuf_tensor("partial_sb", [P, 1], FT))
        ones = ctx.enter_context(nc.sbuf_tensor("ones_sb", [P, 1], FT))
        scal = ctx.enter_context(nc.sbuf_tensor("scal_sb", [1, 1], FT))
        scal_ps = ctx.enter_context(nc.psum_tensor("scal_ps", [1, 1], FT))

        s_slot = [ctx.enter_context(nc.semaphore(f"s_slot{i}")) for i in range(NSLOT)]
        s_gin = ctx.enter_context(nc.semaphore("s_gin"))
        s_wt = ctx.enter_context(nc.semaphore("s_wt"))
        s_cst = ctx.enter_context(nc.semaphore("s_cst"))
        s_g = ctx.enter_context(nc.semaphore("s_g"))
        s_exp = ctx.enter_context(nc.semaphore("s_exp"))
        s_red = ctx.enter_context(nc.semaphore("s_red"))
        s_sml = ctx.enter_context(nc.semaphore("s_sml"))
        s_ln = ctx.enter_context(nc.semaphore("s_ln"))
        s_fin = ctx.enter_context(nc.semaphore("s_fin"))
        s_par = ctx.enter_context(nc.semaphore("s_par"))
        s_ones = ctx.enter_context(nc.semaphore("s_ones"))
        s_mm = ctx.enter_context(nc.semaphore("s_mm"))
        s_dve = ctx.enter_context(nc.semaphore("s_dve"))
        s_out = ctx.enter_context(nc.semaphore("s_out"))

        x_tiles = x[:].rearrange("(n p q) c -> n p (q c)", p=P, q=Q)
        # strided views of the select columns of xg for the
        # ln(exp(x_tgt)) -> x_tgt fixup, covering exactly SEL_COLS:
        # odd tiles 1,3..25 -> cols 4+8a (a=0..12); even select tiles
        # 2,6..22 -> cols 8+16a (a=0..5). Gathered columns untouched.
        xg_sel_a = xg[:].rearrange("p (a t b) -> p a t b", t=2, b=Q)[
            :, 0:13, 1, 0
        ]
        xg_sel_b = xg[:].rearrange("p (a s b) -> p a s b", s=4, b=Q)[
            :, 0:6, 2, 0
        ]

        with nc.Block() as block:

            @block.sync
            def _(sync):
                for j in range(NBIG):
                    if j >= NSLOT:
                        sync.wait_ge(s_red, j - NSLOT + 1)
                    sync.dma_start(
                        out=xbuf[j % NSLOT][:], in_=x_tiles[j]
                    ).then_inc(s_slot[j % NSLOT], 16)
                for h in range(4):
                    jj = NBIG + h
                    sync.wait_ge(s_red, jj - NSLOT + 1)
                    sync.dma_start(
                        out=xbuf[jj % NSLOT][:, 0:C],
                        in_=x_tiles[NBIG][:, h * C:(h + 1) * C],
                    ).then_inc(s_slot[jj % NSLOT], 16)
                sync.wait_ge(s_dve, 1)
                sync.dma_start(out=out[:], in_=scal[:]).then_inc(s_out, 16)
                sync.wait_ge(s_out, 16)

            @block.scalar
            def _(scalar):
                scalar.dma_start(out=wtt[:], in_=wt[:]).then_inc(s_wt, 16)
                scalar.dma_start(out=tgtft[:], in_=tgtf[:]).then_inc(s_cst, 16)
                scalar.dma_start(out=iotat[:], in_=iotaf[:]).then_inc(s_cst, 16)
                # ones = garbage*0 + 1
                scalar.activation(
                    out=ones[:], in_=ones[:], func=Ident, bias=1.0, scale=0.0
                ).then_inc(s_ones, 1)
                for j in range(NBIG):
                    scalar.wait_ge(s_slot[j % NSLOT], 16 * (j // NSLOT + 1))
                    scalar.activation(
                        out=xbuf[j % NSLOT][:], in_=xbuf[j % NSLOT][:], func=Exp
                    ).then_inc(s_exp, 1)
                # tail tiles FIRST: they only need their DMAs (landed by
                # ~stream end), while Ln124 waits on reduce30 which ends
                # ~3.7us past stream end — queuing the Lns first was
                # measured to push the small acts (and the whole result
                # chain) ~4.5us later
                for h in range(4):
                    jj = NBIG + h
                    scalar.wait_ge(s_slot[jj % NSLOT], 16 * (jj // NSLOT + 1))
                    scalar.activation(
                        out=xbuf[jj % NSLOT][:, 0:C],
                        in_=xbuf[jj % NSLOT][:, 0:C],
                        func=Exp,
                        accum_out=sums[:, 4 * NBIG + h:4 * NBIG + h + 1],
                    ).then_inc(s_sml, 1)
                scalar.wait_ge(s_red, NBIG)
                # select columns hold exp(x_tgt); fix up to x_tgt first so
                # the s_ln>=1 edge below covers these writes for the DVE
                scalar.activation(out=xg_sel_a, in_=xg_sel_a, func=Ln)
                scalar.activation(out=xg_sel_b, in_=xg_sel_b, func=Ln)
                scalar.activation(
                    out=lse[:, 0:4 * NBIG], in_=sums[:, 0:4 * NBIG], func=Ln
                ).then_inc(s_ln, 1)
                scalar.wait_ge(s_sml, 4)
                scalar.activation(
                    out=lse[:, 4 * NBIG:NCOL], in_=sums[:, 4 * NBIG:NCOL], func=Ln
                ).then_inc(s_ln, 1)

            @block.gpsimd
            def _(gpsimd):
                gpsimd.dma_start(out=gofft[:], in_=goff[:]).then_inc(s_gin, 16)
                gpsimd.wait_ge(s_gin, 16)
                for col in range(NCOL):
                    if col in SEL_COLS:
                        continue
                    gpsimd.indirect_dma_start(
                        out=xg[:, col:col + 1],
                        out_offset=None,
                        in_=x[:],
                        in_offset=bass.IndirectOffsetOnAxis(
                            ap=gofft[:, col:col + 1], axis=1
                        ),
                    ).then_inc(s_g, 16)

            @block.vector
            def _(vector):
                first_sel = True
                for j in range(NBIG):
                    vector.wait_ge(s_exp, j + 1)
                    red = vector.tensor_reduce(
                        out=sums[:, Q * j:Q * j + Q],
                        in_=xbuf[j % NSLOT][:].rearrange("p (q c) -> p q c", q=Q),
                        axis=mybir.AxisListType.X,
                        op=mybir.AluOpType.add,
                    )
                    if j not in SEL_TILES:
                        red.then_inc(s_red, 1)
                    else:
                        if first_sel:
                            vector.wait_ge(s_cst, 32)
                            first_sel = False
                        else:
                            # same-engine WAW on trasht needs explicit
                            # sync (deep DVE pipeline); the previous
                            # select's inc is contained in s_red >= j
                            vector.wait_ge(s_red, j)
                        # accum = sum((iota==tgt)*exp_tile) = exp(x_tgt);
                        # the select is the tile's last reader so it
                        # carries the slot-free inc
                        vector.scalar_tensor_tensor(
                            out=trasht[:],
                            in0=iotat[:],
                            scalar=tgtft[:, Q * j:Q * j + 1],
                            in1=xbuf[j % NSLOT][:, 0:C],
                            op0=mybir.AluOpType.is_equal,
                            op1=mybir.AluOpType.mult,
                            accum_out=xg[:, Q * j:Q * j + 1],
                        ).then_inc(s_red, 1)
                vector.wait_ge(s_ln, 1)
                vector.wait_ge(s_g, 16 * NGATH)
                vector.wait_ge(s_wt, 16)
                vector.tensor_tensor(
                    out=diff[:, 0:4 * NBIG], in0=lse[:, 0:4 * NBIG],
                    in1=xg[:, 0:4 * NBIG], op=mybir.AluOpType.subtract,
                ).then_inc(s_fin, 1)
                vector.wait_ge(s_fin, 1)
                vector.tensor_tensor(
                    out=prod[:, 0:4 * NBIG], in0=diff[:, 0:4 * NBIG],
                    in1=wtt[:, 0:4 * NBIG], op=mybir.AluOpType.mult,
                ).then_inc(s_fin, 1)
                vector.wait_ge(s_ln, 2)
                vector.tensor_tensor(
                    out=diff[:, 4 * NBIG:NCOL], in0=lse[:, 4 * NBIG:NCOL],
                    in1=xg[:, 4 * NBIG:NCOL], op=mybir.AluOpType.subtract,
                ).then_inc(s_fin, 1)
                vector.wait_ge(s_fin, 3)
                vector.tensor_tensor(
                    out=prod[:, 4 * NBIG:NCOL], in0=diff[:, 4 * NBIG:NCOL],
                    in1=wtt[:, 4 * NBIG:NCOL], op=mybir.AluOpType.mult,
                ).then_inc(s_fin, 1)
                vector.wait_ge(s_fin, 4)
                vector.tensor_reduce(
                    out=partial[:],
                    in_=prod[:],
                    axis=mybir.AxisListType.X,
                    op=mybir.AluOpType.add,
                ).then_inc(s_par, 1)
                vector.wait_ge(s_mm, 1)
                vector.tensor_copy(out=scal[:], in_=scal_ps[:]).then_inc(s_dve, 1)

            @block.tensor
            def _(tensor):
                tensor.wait_ge(s_ones, 1)
                tensor.wait_ge(s_par, 1)
                tensor.matmul(
                    scal_ps[:], partial[:], ones[:], start=True, stop=True,
                ).then_inc(s_mm, 1)

    return nc


def _build_bass_raw():
    """Raw-bass (no Tile) variant: manual semaphores, one wait per
    instruction by construction. Saves most of Tile's ~9us end-of-kernel
    drain/barrier tail and some preamble."""
    from contextlib import ExitStack

    nc = bass.Bass()
    x = nc.declare_dram_parameter("x", [TS, C], mybir.dt.float32, isOutput=False)
    goff = nc.declare_dram_parameter("goff", [P, NCOL], mybir.dt.int32, isOutput=False)
    wt = nc.declare_dram_parameter("wt", [P, NCOL], mybir.dt.float32, isOutput=False)
    out = nc.declare_dram_parameter("partial", [P, 1], mybir.dt.float32, isOutput=True)

    FT = mybir.dt.float32
    Exp = mybir.ActivationFunctionType.Exp
    Ln = mybir.ActivationFunctionType.Ln
    NSLOT = 8

    with ExitStack() as ctx:
        xbuf = [
            ctx.enter_context(nc.sbuf_tensor(f"xbuf{i}", [P, Q * C], FT))
            for i in range(NSLOT)
        ]
        gofft = ctx.enter_context(nc.sbuf_tensor("gofft_sb", [P, NCOL], mybir.dt.int32))
        wtt = ctx.enter_context(nc.sbuf_tensor("wtt_sb", [P, NCOL], FT))
        xg = ctx.enter_context(nc.sbuf_tensor("xg_sb", [P, NCOL], FT))
        sums = ctx.enter_context(nc.sbuf_tensor("sums_sb", [P, NCOL], FT))
        lse = ctx.enter_context(nc.sbuf_tensor("lse_sb", [P, NCOL], FT))
        diff = ctx.enter_context(nc.sbuf_tensor("diff_sb", [P, NCOL], FT))
        prod = ctx.enter_context(nc.sbuf_tensor("prod_sb", [P, NCOL], FT))
        partial = ctx.enter_context(nc.sbuf_tensor("partial_sb", [P, 1], FT))

        s_slot = [ctx.enter_context(nc.semaphore(f"s_slot{i}")) for i in range(NSLOT)]
        s_gin = ctx.enter_context(nc.semaphore("s_gin"))
        s_wt = ctx.enter_context(nc.semaphore("s_wt"))
        s_g = ctx.enter_context(nc.semaphore("s_g"))
        s_act = ctx.enter_context(nc.semaphore("s_act"))
        s_red = ctx.enter_context(nc.semaphore("s_red"))
        s_ln = ctx.enter_context(nc.semaphore("s_ln"))
        s_dve = ctx.enter_context(nc.semaphore("s_dve"))
        s_out = ctx.enter_context(nc.semaphore("s_out"))
        s_fin = ctx.enter_context(nc.semaphore("s_fin"))

        x_tiles = x[:].rearrange("(n p q) c -> n p (q c)", p=P, q=Q)

        with nc.Block() as block:

            @block.sync
            def _(sync):
                sync.dma_start(out=gofft[:], in_=goff[:]).then_inc(s_gin, 16)
                sync.dma_start(out=wtt[:], in_=wt[:]).then_inc(s_wt, 16)
                for j in range(NTILES):
                    if j >= NSLOT:
                        sync.wait_ge(s_red, j - NSLOT + 1)
                    sync.dma_start(
                        out=xbuf[j % NSLOT][:], in_=x_tiles[j]
                    ).then_inc(s_slot[j % NSLOT], 16)
                sync.wait_ge(s_dve, 1)
                sync.dma_start(out=out[:], in_=partial[:]).then_inc(s_out, 16)
                sync.wait_ge(s_out, 16)

            @block.gpsimd
            def _(gpsimd):
                gpsimd.wait_ge(s_gin, 16)
                for col in range(NCOL):
                    gpsimd.indirect_dma_start(
                        out=xg[:, col:col + 1],
                        out_offset=None,
                        in_=x[:],
                        in_offset=bass.IndirectOffsetOnAxis(
                            ap=gofft[:, col:col + 1], axis=1
                        ),
                    ).then_inc(s_g, 16)

            @block.scalar
            def _(scalar):
                for j in range(NTILES):
                    scalar.wait_ge(s_slot[j % NSLOT], 16 * (j // NSLOT + 1))
                    scalar.activation(
                        out=xbuf[j % NSLOT][:], in_=xbuf[j % NSLOT][:], func=Exp
                    ).then_inc(s_act, 1)
                scalar.wait_ge(s_red, NTILES)
                scalar.activation(out=lse[:], in_=sums[:], func=Ln).then_inc(s_ln, 1)

            @block.vector
            def _(vector):
                for j in range(NTILES):
                    vector.wait_ge(s_act, j + 1)
                    vector.tensor_reduce(
                        out=sums[:, Q * j:Q * j + Q],
                        in_=xbuf[j % NSLOT][:].rearrange("p (q c) -> p q c", q=Q),
                        axis=mybir.AxisListType.X,
                        op=mybir.AluOpType.add,
                    ).then_inc(s_red, 1)
                vector.wait_ge(s_ln, 1)
                vector.wait_ge(s_g, 16 * NCOL)
                vector.wait_ge(s_wt, 16)
                # same-engine RAW chains need explicit sync (deep pipeline)
                vector.tensor_tensor(
                    out=diff[:], in0=lse[:], in1=xg[:], op=mybir.AluOpType.subtract
                ).then_inc(s_fin, 1)
                vector.wait_ge(s_fin, 1)
                vector.tensor_tensor(
                    out=prod[:], in0=diff[:], in1=wtt[:], op=mybir.AluOpType.mult
                ).then_inc(s_fin, 1)
                vector.wait_ge(s_fin, 2)
                vector.tensor_reduce(
                    out=partial[:],
                    in_=prod[:],
                    axis=mybir.AxisListType.X,
                    op=mybir.AluOpType.add,
                ).then_inc(s_dve, 1)

    return nc


def _legalize_waits(nc):
    """This walrus build accepts at most 1 semaphore wait per instruction
    (2 for EventSemaphore — see bass_rust.inst_waits_full), but Tile's wait
    assignment attaches more. Spill excess waits onto standalone
    EventSemaphore instructions (what raw-bass wait_ge emits) inserted just
    before the over-full instruction on the same engine, then pin the
    legalized JSON onto nc.to_json_bytes so both the native compile path and
    the bass2jax/PJRT path use it."""
    obj = json.loads(nc.to_json_bytes())
    n_new = 0
    for fn in obj["functions"]:
        for bb in fn["blocks"]:
            insts = bb["instructions"]
            out = []
            for inst in insts:
                si = inst.get("sync_info")
                waits = (si or {}).get("on_wait") or []
                cap = 2 if inst.get("opcode") == "EventSemaphore" else 1
                if len(waits) > cap:
                    excess, keep = waits[:-cap], waits[-cap:]
                    si["on_wait"] = keep
                    for k in range(0, len(excess), 2):
                        out.append(
                            {
                                "engine": inst["engine"],
                                "ins": [],
                                "name": f"EVSPLIT-{n_new}",
                                "opcode": "EventSemaphore",
                                "outs": [],
                                "sync_info": {
                                    "on_update": [],
                                    "on_wait": excess[k:k + 2],
                                },
                            }
                        )
                        n_new += 1
                out.append(inst)
            bb["instructions"] = out
    legal = json.dumps(obj).encode()
    nc.to_json_bytes = lambda: legal
    return n_new


def _host_weights(lengths: np.ndarray, gamma: float) -> np.ndarray:
    """Per-token weights w[t]: segment softmax of linspace(-g, g, L_seg)."""
    lengths = lengths.astype(np.int64)
    seg = np.repeat(np.arange(B), lengths)
    starts = np.cumsum(lengths) - lengths
    pos = np.arange(T, dtype=np.int64) - starts[seg]
    Ls = lengths[seg]
    g = np.float32(gamma)
    denom = np.maximum(Ls - 1, 1).astype(np.float32)
    raw = (-g + (np.float32(2.0) * g) * pos.astype(np.float32) / denom).astype(
        np.float32
    )
    e = np.exp(raw - g).astype(np.float32)
    ssum = np.zeros(B, np.float32)
    np.add.at(ssum, seg, e)
    return (e / ssum[seg]).astype(np.float32)


def kernel(outputs, targets, lengths, gamma):
    global _cached, last_results
    x = np.ascontiguousarray(np.asarray(outputs), dtype=np.float32)
    tgt = np.asarray(targets).astype(np.int64)
    lens = np.asarray(lengths).astype(np.int64)
    g = float(np.asarray(gamma))

    w = _host_weights(lens, g)

    # [p, col] -> local token index: t_loc = 256*(col//Q) + Q*p + (col%Q)
    cols = np.arange(NCOL, dtype=np.int64)
    ps = np.arange(P, dtype=np.int64)[:, None]
    t_loc = (P * Q) * (cols // Q) + Q * ps + (cols % Q)  # [P, NCOL]

    in_maps = []
    if VARIANT == "v3":
        # ap_gather idx table: [p, col] = (col%Q)*C + tgt[t_loc] for the
        # big tiles; the 4 tail columns index a [128, C] source directly.
        # mask[p, r*16+s] = 1 iff s == p%16 (selects the per-partition
        # diagonal of each 16-wide gather group).
        mask = np.zeros((P, 64), np.float32)
        for r in range(4):
            mask[np.arange(P), 16 * r + (np.arange(P) % 16)] = 1.0
        for c in range(NCORES):
            lo = c * TS
            tgt_l = tgt[lo:lo + TS]
            w_l = w[lo:lo + TS]
            gidx_c = ((cols % Q) * C + tgt_l[t_loc]).astype(np.int16)
            gidx_c[:, 4 * (NTILES - 1):] = tgt_l[t_loc[:, 4 * (NTILES - 1):]]
            wt_c = w_l[t_loc].astype(np.float32)
            in_maps.append(
                {
                    "x": x[lo:lo + TS],
                    "gidx": np.ascontiguousarray(gidx_c),
                    "mask": mask,
                    "wt": np.ascontiguousarray(wt_c),
                }
            )
    else:
        iota_f = np.broadcast_to(
            np.arange(C, dtype=np.float32), (P, C)
        ).copy() if VARIANT == "v5" else None
        for c in range(NCORES):
            lo = c * TS
            tgt_l = tgt[lo:lo + TS]
            w_l = w[lo:lo + TS]
            goff_c = (t_loc * C + tgt_l[t_loc]).astype(np.int32)
            wt_c = w_l[t_loc].astype(np.float32)
            m = {
                "x": x[lo:lo + TS],
                "goff": np.ascontiguousarray(goff_c),
                "wt": np.ascontiguousarray(wt_c),
            }
            if VARIANT == "v5":
                m["tgtf"] = np.ascontiguousarray(tgt_l[t_loc].astype(np.float32))
                m["iotaf"] = iota_f
            in_maps.append(m)

    if _cached is None:
        builder = {
            "tile": _build_bass,
            "raw": _build_bass_raw,
            "v2": _build_bass_v2,
            "v3": _build_bass_v3,
            "v4": _build_bass_v4,
            "v5": _build_bass_v5,
        }[VARIANT]
        nc = builder()
        _legalize_waits(nc)
        _cached = nc
    nc = _cached

    def _run():
        return run_bass_kernel_spmd(nc, in_maps, core_ids=list(range(NCORES)))

    try:
        last_results = _run()
    except ModuleNotFoundError:
        # BASS_TRACE requested under axon but the image lacks
        # antenv.axon_hooks — rerun without tracing.
        _os.environ["BASS_NEVER_TRACE"] = "1"
        last_results = _run()
    except Exception:
        # transient device errors (e.g. NRT_EXEC_UNIT_UNRECOVERABLE) have
        # been observed on this fabric; retry once after a short pause
        import time as _time

        _time.sleep(5)
        last_results = _run()
    total = np.float64(0.0)
    for r in last_results.results:
        total += np.asarray(r["partial"], dtype=np.float64).sum()
    return np.float32(total / B)

